# revision 7
# baseline (speedup 1.0000x reference)
"""Dilated GRU (3 layers, dilations 1/7/28) Trainium2 Bass kernel.

Strategy: data-parallel over batch (256 -> 32 per core x 8 cores).
Everything on-device runs in a transposed layout: activations are stored
as [hidden, time*batch] with hidden split into 128-partition chunks, so
GRU matmuls are weight-stationary (lhsT = W.T tiles [128,128], moving
operand = h columns).

Per layer:
  gi = x @ W_ih.T is a bulk matmul over all timesteps (chunked by 14
  steps = 448 columns).
  Layer 0 (dilation 1) is a true 168-step recurrence.
  Layers 1/2 only carry state across steps t % dil == 0; all other steps
  use h_prev = 0 and are computed in bulk as elementwise work. The short
  chains (24 / 6 steps) run as small N=32 matmul steps.

Heads (res/day/week), per-(b,t) sum and sum-of-squares of h2 are
reductions over hidden -> done as M=1 matmul passes. Final tanh/bias/
variance arithmetic is O(B*T) scalar work done on host.

Matmul inputs are fp16 (fp32 gate state is carried on-device), gi
buffers are fp16 in SBUF.
"""

import numpy as np
import ml_dtypes

B_FULL = 256
T = 168
NI = 64
H = 512
G3 = 3 * H  # 1536
NCORES = 8
BC = B_FULL // NCORES  # 32
CH = 14                # timesteps per chunk
NCH = T // CH          # 12 chunks
COLS = CH * BC         # 448 columns per chunk
COLS_T = T * BC        # 5376 columns per core
KC = 4                 # 512 / 128 K-chunks
MT = 12                # 1536 / 128 M-tiles
DIL1, DIL2 = 7, 28

_RUNNER = None


def _build_runner():
    import concourse.bass as bass
    import concourse.tile as tile
    import concourse.mybir as mybir
    from concourse import bacc
    from concourse.bass_utils import run_bass_kernel_spmd

    dt = mybir.dt
    AF = mybir.ActivationFunctionType
    OP = mybir.AluOpType

    nc = bacc.Bacc("TRN2", target_bir_lowering=False, debug=False)

    # ---- DRAM I/O ----
    d_xt = nc.dram_tensor("xt", [NI, COLS_T], dt.float16, kind="ExternalInput")
    d_wih0 = nc.dram_tensor("wih0", [NI, G3], dt.float16, kind="ExternalInput")
    d_w = {}
    for name in ("whh0", "wih1", "whh1", "wih2", "whh2"):
        d_w[name] = nc.dram_tensor(name, [H, G3], dt.float16, kind="ExternalInput")
    d_ba = [nc.dram_tensor(f"ba{l}", [128, MT], dt.float32, kind="ExternalInput")
            for l in range(3)]
    d_bhn = [nc.dram_tensor(f"bhn{l}", [128, KC], dt.float32, kind="ExternalInput")
             for l in range(3)]
    d_heads = nc.dram_tensor("heads", [H, 4], dt.float16, kind="ExternalInput")
    # rows of stats output: 0=res logit, 1=day logit, 2=week logit, 3=sum h2, 4=sumsq h2
    d_stats = nc.dram_tensor("stats", [5, COLS_T], dt.float32, kind="ExternalOutput")

    from contextlib import ExitStack

    with tile.TileContext(nc) as tc, ExitStack() as ctx:
        ec = ctx.enter_context
        cw = ec(tc.tile_pool(name="cw", bufs=1))
        xpool = ec(tc.tile_pool(name="xp", bufs=2))
        gi0p = ec(tc.tile_pool(name="gi0", bufs=1))
        gi12p = ec(tc.tile_pool(name="gi12", bufs=2))
        h0p = ec(tc.tile_pool(name="h0r", bufs=3))
        h1p = ec(tc.tile_pool(name="h1r", bufs=3))
        h2p = ec(tc.tile_pool(name="h2r", bufs=3))
        sqp = ec(tc.tile_pool(name="sqp", bufs=2))
        stp = ec(tc.tile_pool(name="stg", bufs=2))
        # small per-step tiles (tags shared across recurrence/chain call sites)
        rzp = ec(tc.tile_pool(name="rzp", bufs=3))
        sgp = ec(tc.tile_pool(name="sgp", bufs=3))
        gp = ec(tc.tile_pool(name="gp", bufs=3))
        hsp = ec(tc.tile_pool(name="hsp", bufs=3))
        bigp = ec(tc.tile_pool(name="bigp", bufs=2))
        ps_rz = ec(tc.tile_pool(name="ps_rz", bufs=2, space="PSUM"))
        ps_n = ec(tc.tile_pool(name="ps_n", bufs=2, space="PSUM"))
        ps_blk = ec(tc.tile_pool(name="ps_blk", bufs=3, space="PSUM"))
        ps_st = ec(tc.tile_pool(name="ps_st", bufs=1, space="PSUM"))

        if True:
            # ---- load constants ----
            w_ih0 = cw.tile([NI, G3], dt.float16, tag="wih0")
            nc.sync.dma_start(out=w_ih0[:], in_=d_wih0[:])
            ws = {}
            for name in ("whh0", "wih1", "whh1", "wih2", "whh2"):
                t_ = cw.tile([128, KC, G3], dt.float16, tag=name)
                nc.sync.dma_start(
                    out=t_[:], in_=d_w[name][:].rearrange("(kc p) m -> p kc m", p=128))
                ws[name] = t_
            ba = []
            bhn = []
            for l in range(3):
                tb = cw.tile([128, MT], dt.float32, tag=f"ba{l}")
                nc.sync.dma_start(out=tb[:], in_=d_ba[l][:])
                ba.append(tb)
                th = cw.tile([128, KC], dt.float32, tag=f"bhn{l}")
                nc.sync.dma_start(out=th[:], in_=d_bhn[l][:])
                bhn.append(th)
            w_heads = cw.tile([128, KC, 4], dt.float16, tag="wheads")
            nc.sync.dma_start(
                out=w_heads[:], in_=d_heads[:].rearrange("(kc p) o -> p kc o", p=128))

            # rolling chunk handles
            h0c, h1c, h2c = {}, {}, {}
            # fp32 carried states
            state = {"l0": None, "l1": None, "l2": None}

            def gi_bulk(wt, rhs_tile, gi_tile, bias_tile, kcs):
                """gi_tile[:, m, :] = bias[:, m] + sum_kc wt[:,kc,m*128..].T @ rhs"""
                for m in range(MT):
                    ps = ps_blk.tile([128, COLS], dt.float32, tag="blk")
                    for k in range(kcs):
                        if kcs == 1:
                            lhsT = wt[:, m * 128:(m + 1) * 128]
                            rhs = rhs_tile[:, :]
                        else:
                            lhsT = wt[:, k, m * 128:(m + 1) * 128]
                            rhs = rhs_tile[:, k, :]
                        nc.tensor.matmul(ps[:], lhsT, rhs,
                                         start=(k == 0), stop=(k == kcs - 1))
                    nc.vector.tensor_scalar_add(
                        out=gi_tile[:, m, :], in0=ps[:], scalar1=bias_tile[:, m:m + 1])

            def gru_step(wname, bhn_tile, gi_tile, lt, rhs_src, rhs_lt,
                         hprev, hout_tile):
                """One GRU step (N=32 cols). rhs_src/rhs_lt: bf16 h_prev tile+col
                (None for a fresh step, h_prev = 0). hprev: fp32 state tile or
                None (falls back to the bf16 rhs columns). Writes bf16 h into
                hout_tile[:, :, lt*32:...]; returns new fp32 state tile."""
                gi_rz = gi_tile[:, 0:8, lt * 32:(lt + 1) * 32]
                gi_n = gi_tile[:, 8:12, lt * 32:(lt + 1) * 32]
                fresh = rhs_src is None
                if not fresh:
                    if hprev is None:
                        hprev = rhs_src[:, :, rhs_lt * 32:(rhs_lt + 1) * 32]
                    prz = ps_rz.tile([128, 8, 32], dt.float32, tag="rz")
                    pn = ps_n.tile([128, 4, 32], dt.float32, tag="n")
                    # m-major: accumulation groups must stay contiguous per
                    # PSUM region (interleaved groups accumulate incorrectly)
                    for m in range(MT):
                        out = prz[:, m, :] if m < 8 else pn[:, m - 8, :]
                        for k in range(KC):
                            rhs = rhs_src[:, k, rhs_lt * 32:(rhs_lt + 1) * 32]
                            nc.tensor.matmul(out, ws[wname][:, k, m * 128:(m + 1) * 128],
                                             rhs, start=(k == 0), stop=(k == KC - 1))
                rzs = sgp.tile([128, 8, 32], dt.float32, tag="rzs")
                omz = sgp.tile([128, 4, 32], dt.float32, tag="omz")
                if fresh:
                    nc.scalar.activation(out=rzs[:], in_=gi_rz, func=AF.Sigmoid)
                    nc.scalar.activation(out=omz[:],
                                         in_=gi_tile[:, 4:8, lt * 32:(lt + 1) * 32],
                                         func=AF.Sigmoid, scale=-1.0)
                else:
                    trz = rzp.tile([128, 8, 32], dt.float32, tag="trz")
                    nc.vector.tensor_add(out=trz[:], in0=prz[:], in1=gi_rz)
                    nc.scalar.activation(out=rzs[:], in_=trz[:], func=AF.Sigmoid)
                    nc.scalar.activation(out=omz[:], in_=trz[:, 4:8, :],
                                         func=AF.Sigmoid, scale=-1.0)
                if hprev is not None:
                    zh = gp.tile([128, 4, 32], dt.float32, tag="zh")
                    nc.vector.tensor_mul(out=zh[:], in0=rzs[:, 4:8, :], in1=hprev[:])
                t3 = gp.tile([128, 4, 32], dt.float32, tag="t3")
                if fresh:
                    for j in range(KC):
                        nc.vector.scalar_tensor_tensor(
                            out=t3[:, j, :], in0=rzs[:, j, :],
                            scalar=bhn_tile[:, j:j + 1],
                            in1=gi_tile[:, 8 + j, lt * 32:(lt + 1) * 32],
                            op0=OP.mult, op1=OP.add)
                else:
                    t2 = gp.tile([128, 4, 32], dt.float32, tag="t2")
                    for j in range(KC):
                        nc.vector.scalar_tensor_tensor(
                            out=t2[:, j, :], in0=pn[:, j, :],
                            scalar=bhn_tile[:, j:j + 1], in1=rzs[:, j, :],
                            op0=OP.add, op1=OP.mult)
                    nc.vector.tensor_add(out=t3[:], in0=t2[:], in1=gi_n)
                nt = gp.tile([128, 4, 32], dt.float32, tag="nt")
                nc.scalar.activation(out=nt[:], in_=t3[:], func=AF.Tanh)
                hnew = hsp.tile([128, 4, 32], dt.float32, tag="h")
                if hprev is not None:
                    u = gp.tile([128, 4, 32], dt.float32, tag="u")
                    nc.vector.tensor_mul(out=u[:], in0=nt[:], in1=omz[:])
                    nc.vector.tensor_add(out=hnew[:], in0=u[:], in1=zh[:])
                else:
                    nc.vector.tensor_mul(out=hnew[:], in0=nt[:], in1=omz[:])
                nc.gpsimd.tensor_copy(out=hout_tile[:, :, lt * 32:(lt + 1) * 32],
                                      in_=hnew[:])
                return hnew

            def parallel_gates(gi_tile, bhn_tile, hout_tile):
                """Bulk h = (1-z)*n for all CH steps of a chunk (h_prev = 0)."""
                rzb = bigp.tile([128, 8, COLS], dt.float16, tag="rzb")
                nc.scalar.activation(out=rzb[:], in_=gi_tile[:, 0:8, :], func=AF.Sigmoid)
                t2b = bigp.tile([128, 4, COLS], dt.float16, tag="t2b")
                for j in range(KC):
                    nc.vector.scalar_tensor_tensor(
                        out=t2b[:, j, :], in0=rzb[:, j, :],
                        scalar=bhn_tile[:, j:j + 1], in1=gi_tile[:, 8 + j, :],
                        op0=OP.mult, op1=OP.add)
                nb = bigp.tile([128, 4, COLS], dt.float16, tag="nb")
                nc.scalar.activation(out=nb[:], in_=t2b[:], func=AF.Tanh)
                znb = bigp.tile([128, 4, COLS], dt.float16, tag="znb")
                nc.vector.tensor_mul(out=znb[:], in0=rzb[:, 4:8, :], in1=nb[:])
                nc.vector.tensor_sub(out=hout_tile[:], in0=nb[:], in1=znb[:])

            def head_pass(col, rhs_tile, stage, row):
                """stats row = w_heads[:, col] . rhs  (M=1 matmul, 4 K-chunks)."""
                ps = ps_st.tile([1, COLS], dt.float32, tag="st")
                for k in range(KC):
                    nc.tensor.matmul(ps[:], w_heads[:, k, col:col + 1],
                                     rhs_tile[:, k, :], start=(k == 0), stop=(k == KC - 1))
                nc.vector.tensor_copy(out=stage[:, row, :], in_=ps[:])

            # ================= main pipeline =================
            for slot in range(NCH + 2):
                c = slot
                # ---- gi0(c) + layer-0 recurrence chunk c ----
                if c < NCH:
                    xt = xpool.tile([NI, COLS], dt.float16, tag="xt")
                    nc.sync.dma_start(out=xt[:], in_=d_xt[:, c * COLS:(c + 1) * COLS])
                    gi0 = gi0p.tile([128, MT, COLS], dt.float16, tag="gi0")
                    gi_bulk(w_ih0, xt, gi0, ba[0], 1)
                    hc = h0p.tile([128, KC, COLS], dt.float16, tag="h0c")
                    h0c[c] = hc
                    for lt in range(CH):
                        t = c * CH + lt
                        if t == 0:
                            rhs_src, rhs_lt = None, 0
                        elif lt == 0:
                            rhs_src, rhs_lt = h0c[c - 1], CH - 1
                        else:
                            rhs_src, rhs_lt = hc, lt - 1
                        state["l0"] = gru_step("whh0", bhn[0], gi0, lt,
                                               rhs_src, rhs_lt, state["l0"], hc)

                # ---- layer 1 on chunk c-1 ----
                c1 = slot - 1
                if 0 <= c1 < NCH:
                    gi1 = gi12p.tile([128, MT, COLS], dt.float16, tag="gi12")
                    gi_bulk(ws["wih1"], h0c[c1], gi1, ba[1], KC)
                    hc = h1p.tile([128, KC, COLS], dt.float16, tag="h1c")
                    h1c[c1] = hc
                    parallel_gates(gi1, bhn[1], hc)
                    # chain steps t % 7 == 0 (t>0) in this chunk: lt = 0, 7
                    for lt in (0, 7):
                        t = c1 * CH + lt
                        if t == 0:
                            continue
                        tp = t - DIL1
                        cp, lp = tp // CH, tp % CH
                        state["l1"] = gru_step("whh1", bhn[1], gi1, lt,
                                               h1c[cp], lp, state["l1"], hc)

                # ---- layer 2 + stats on chunk c-2 ----
                c2 = slot - 2
                if 0 <= c2 < NCH:
                    gi2 = gi12p.tile([128, MT, COLS], dt.float16, tag="gi12")
                    gi_bulk(ws["wih2"], h1c[c2], gi2, ba[2], KC)
                    hc = h2p.tile([128, KC, COLS], dt.float16, tag="h2c")
                    h2c[c2] = hc
                    parallel_gates(gi2, bhn[2], hc)
                    if c2 % 2 == 0 and c2 > 0:
                        t = c2 * CH  # t = 28k, lt = 0
                        tp = t - DIL2
                        cp, lp = tp // CH, tp % CH
                        state["l2"] = gru_step("whh2", bhn[2], gi2, 0,
                                               h2c[cp], lp, state["l2"], hc)
                    # stats / heads
                    stage = stp.tile([1, 5, COLS], dt.float32, tag="stage")
                    head_pass(0, h0c[c2], stage, 0)   # res  . h0
                    head_pass(1, h1c[c2], stage, 1)   # day  . h1
                    head_pass(2, hc, stage, 2)        # week . h2
                    head_pass(3, hc, stage, 3)        # ones . h2
                    sq = sqp.tile([128, KC, COLS], dt.float16, tag="sq")
                    nc.vector.tensor_mul(out=sq[:], in0=hc[:], in1=hc[:])
                    head_pass(3, sq, stage, 4)        # ones . h2^2
                    nc.sync.dma_start(out=d_stats[:, c2 * COLS:(c2 + 1) * COLS],
                                      in_=stage[:])

    nc.compile()

    # ---- build a cached jitted executor (mirrors bass2jax.run_bass_via_pjrt,
    # but the jit closure is constructed once so repeat calls skip retracing) ----
    import jax
    from jax.experimental.shard_map import shard_map
    from jax.sharding import Mesh, PartitionSpec, NamedSharding
    from concourse.bass2jax import (_bass_exec_p, install_neuronx_cc_hook,
                                    partition_id_tensor)

    install_neuronx_cc_hook()
    partition_name = nc.partition_id_tensor.name if nc.partition_id_tensor else None
    in_names, out_names, out_avals, zero_outs = [], [], [], []
    for alloc in nc.m.functions[0].allocations:
        if not isinstance(alloc, mybir.MemoryLocationSet):
            continue
        name = alloc.memorylocations[0].name
        if alloc.kind == "ExternalInput":
            if name != partition_name:
                in_names.append(name)
        elif alloc.kind == "ExternalOutput":
            out_names.append(name)
            shape = tuple(alloc.tensor_shape)
            npdt = mybir.dt.np(alloc.dtype)
            out_avals.append(jax.core.ShapedArray(shape, npdt))
            zero_outs.append(np.zeros(shape, npdt))
    n_params, n_outs = len(in_names), len(out_names)
    all_names = in_names + out_names + ([partition_name] if partition_name else [])

    def _body(*args):
        operands = list(args)
        if partition_name is not None:
            operands.append(partition_id_tensor())
        outs = _bass_exec_p.bind(
            *operands, out_avals=tuple(out_avals), in_names=tuple(all_names),
            out_names=tuple(out_names), lowering_input_output_aliases=(),
            sim_require_finite=True, sim_require_nnan=True, nc=nc)
        return tuple(outs)

    try:
        devices = jax.devices("axon")[:NCORES]
    except Exception:
        devices = jax.devices()[:NCORES]
    assert len(devices) == NCORES, f"need {NCORES} neuron cores, got {devices}"
    mesh = Mesh(np.asarray(devices), ("core",))
    sharding = NamedSharding(mesh, PartitionSpec("core"))
    donate = tuple(range(n_params, n_params + n_outs))
    sharded = jax.jit(
        shard_map(_body, mesh=mesh,
                  in_specs=(PartitionSpec("core"),) * (n_params + n_outs),
                  out_specs=(PartitionSpec("core"),) * n_outs,
                  check_rep=False),
        donate_argnums=donate, keep_unused=True)

    def _concat_inputs(in_maps):
        return [np.concatenate([np.asarray(m[name]) for m in in_maps], axis=0)
                for name in in_names]

    def _zeros():
        return [np.zeros((NCORES * z.shape[0], *z.shape[1:]), z.dtype)
                for z in zero_outs]

    def run(in_maps):
        out_arrs = sharded(*_concat_inputs(in_maps), *_zeros())
        stats = np.asarray(out_arrs[out_names.index("stats")])
        return list(stats.reshape(NCORES, 5, COLS_T))

    def time_it(in_maps, iters=10):
        import time
        dev_in = [jax.device_put(a, sharding) for a in _concat_inputs(in_maps)]
        jax.block_until_ready(sharded(*dev_in, *_zeros()))  # warm
        times = []
        for _ in range(iters):
            zs = _zeros()
            t0 = time.perf_counter()
            jax.block_until_ready(sharded(*dev_in, *zs))
            times.append(time.perf_counter() - t0)
        return times

    run.time_it = time_it
    return run


def _prep_inputs(inputs):
    f16 = np.float16
    x = np.asarray(inputs["x"], np.float32)          # [256, 168, 64]
    maps = []
    shared = {}
    for l in range(3):
        wih = np.asarray(inputs[f"W_ih{l}"], np.float32)   # [1536, in]
        whh = np.asarray(inputs[f"W_hh{l}"], np.float32)   # [1536, 512]
        bih = np.asarray(inputs[f"b_ih{l}"], np.float32)
        bhh = np.asarray(inputs[f"b_hh{l}"], np.float32)
        shared[f"wih{l}" if l else "wih0"] = np.ascontiguousarray(wih.T).astype(f16)
        shared[f"whh{l}"] = np.ascontiguousarray(whh.T).astype(f16)      # [512,1536]
        bias_a = bih.copy()
        bias_a[:2 * H] += bhh[:2 * H]          # rz tiles: b_ih + b_hh ; n tiles: b_ih
        shared[f"ba{l}"] = np.ascontiguousarray(bias_a.reshape(MT, 128).T)  # [128,12]
        shared[f"bhn{l}"] = np.ascontiguousarray(bhh[2 * H:].reshape(KC, 128).T)
    heads = np.zeros((H, 4), np.float32)
    heads[:, 0] = np.asarray(inputs["w_res"], np.float32)[0]
    heads[:, 1] = np.asarray(inputs["w_day"], np.float32)[0]
    heads[:, 2] = np.asarray(inputs["w_week"], np.float32)[0]
    heads[:, 3] = 1.0
    shared["heads"] = heads.astype(f16)

    for k in range(NCORES):
        xc = x[k * BC:(k + 1) * BC]                        # [32, 168, 64]
        xt = np.ascontiguousarray(xc.transpose(2, 1, 0).reshape(NI, COLS_T))
        m = dict(shared)
        m["xt"] = xt.astype(f16)
        maps.append(m)
    return maps


def kernel(**inputs):
    global _RUNNER
    if _RUNNER is None:
        _RUNNER = _build_runner()
    maps = _prep_inputs(inputs)
    stats = _RUNNER(maps)          # list of [5, 5376] per core

    b_res = float(np.asarray(inputs["b_res"], np.float32)[0])
    b_day = float(np.asarray(inputs["b_day"], np.float32)[0])
    b_week = float(np.asarray(inputs["b_week"], np.float32)[0])

    outputs = np.zeros((B_FULL, T), np.float32)
    aux = 0.0
    for k in range(NCORES):
        s = np.asarray(stats[k], np.float64)               # [5, 5376]
        res, day, week, sm, sq = [s[i].reshape(T, BC) for i in range(5)]
        out = np.tanh(res + b_res) + np.tanh(day + b_day) + np.tanh(week + b_week)
        outputs[k * BC:(k + 1) * BC, :] = out.T.astype(np.float32)
        var = (sq - sm * sm / H) / (H - 1)
        aux += var.sum()
    aux = np.float32(aux / B_FULL)
    return outputs, aux


# revision 15
# speedup vs baseline: 1.2866x; 1.2866x over previous
"""Dilated GRU (3 layers, dilations 1/7/28) Trainium2 Bass kernel.

Strategy: data-parallel over batch (256 -> 32 per core x 8 cores).
Everything on-device runs in a transposed layout: activations are stored
as [hidden, time*batch] with hidden split into 128-partition chunks, so
GRU matmuls are weight-stationary (lhsT = W.T tiles [128,128], moving
operand = h columns).

Per layer:
  gi = x @ W_ih.T is a bulk matmul over all timesteps (chunked by 14
  steps = 448 columns).
  Layer 0 (dilation 1) is a true 168-step recurrence.
  Layers 1/2 only carry state across steps t % dil == 0; all other steps
  use h_prev = 0 and are computed in bulk as elementwise work. The short
  chains (24 / 6 steps) run as small N=32 matmul steps.

Heads (res/day/week), per-(b,t) sum and sum-of-squares of h2 are
reductions over hidden -> done as M=1 matmul passes. Final tanh/bias/
variance arithmetic is O(B*T) scalar work done on host.

Matmul inputs are fp16 (fp32 gate state is carried on-device), gi
buffers are fp16 in SBUF.
"""

import numpy as np
import ml_dtypes

B_FULL = 256
T = 168
NI = 64
H = 512
G3 = 3 * H  # 1536
NCORES = 8
BC = B_FULL // NCORES  # 32
CH = 14                # timesteps per chunk
NCH = T // CH          # 12 chunks
COLS = CH * BC         # 448 columns per chunk
COLS_T = T * BC        # 5376 columns per core
KC = 4                 # 512 / 128 K-chunks
MT = 12                # 1536 / 128 M-tiles
DIL1, DIL2 = 7, 28

_RUNNER = None


def _build_runner():
    import concourse.bass as bass
    import concourse.tile as tile
    import concourse.mybir as mybir
    from concourse import bacc
    from concourse.bass_utils import run_bass_kernel_spmd

    dt = mybir.dt
    AF = mybir.ActivationFunctionType
    OP = mybir.AluOpType

    nc = bacc.Bacc("TRN2", target_bir_lowering=False, debug=False)

    # ---- DRAM I/O ----
    d_xt = nc.dram_tensor("xt", [NI, COLS_T], dt.float16, kind="ExternalInput")
    d_wih0 = nc.dram_tensor("wih0", [NI, G3], dt.float16, kind="ExternalInput")
    d_w = {}
    for name in ("whh0", "wih1", "whh1", "wih2", "whh2"):
        d_w[name] = nc.dram_tensor(name, [H, G3], dt.float16, kind="ExternalInput")
    d_ba = [nc.dram_tensor(f"ba{l}", [128, MT], dt.float32, kind="ExternalInput")
            for l in range(3)]
    d_bhn = [nc.dram_tensor(f"bhn{l}", [128, KC], dt.float32, kind="ExternalInput")
             for l in range(3)]
    d_bhnb = [nc.dram_tensor(f"bhnb{l}", [128, KC, 32], dt.float32, kind="ExternalInput")
              for l in range(3)]
    d_heads = nc.dram_tensor("heads", [H, 4], dt.float16, kind="ExternalInput")
    # rows of stats output: 0=res logit, 1=day logit, 2=week logit, 3=sum h2, 4=sumsq h2
    d_stats = nc.dram_tensor("stats", [5, COLS_T], dt.float32, kind="ExternalOutput")

    from contextlib import ExitStack

    with tile.TileContext(nc) as tc, ExitStack() as ctx:
        ec = ctx.enter_context
        cw = ec(tc.tile_pool(name="cw", bufs=1))
        xpool = ec(tc.tile_pool(name="xp", bufs=2))
        gi0p = ec(tc.tile_pool(name="gi0", bufs=1))
        gi12p = ec(tc.tile_pool(name="gi12", bufs=2))
        h0p = ec(tc.tile_pool(name="h0r", bufs=3))
        h1p = ec(tc.tile_pool(name="h1r", bufs=3))
        h2p = ec(tc.tile_pool(name="h2r", bufs=3))
        sqp = ec(tc.tile_pool(name="sqp", bufs=2))
        stp = ec(tc.tile_pool(name="stg", bufs=2))
        # small per-step tiles (tags shared across recurrence/chain call sites)
        rzp = ec(tc.tile_pool(name="rzp", bufs=3))
        sgp = ec(tc.tile_pool(name="sgp", bufs=3))
        gp = ec(tc.tile_pool(name="gp", bufs=3))
        hsp = ec(tc.tile_pool(name="hsp", bufs=3))
        bigp = ec(tc.tile_pool(name="bigp", bufs=2))
        ps_rz = ec(tc.tile_pool(name="ps_rz", bufs=2, space="PSUM"))
        ps_n = ec(tc.tile_pool(name="ps_n", bufs=2, space="PSUM"))
        ps_blk = ec(tc.tile_pool(name="ps_blk", bufs=3, space="PSUM"))
        ps_st = ec(tc.tile_pool(name="ps_st", bufs=1, space="PSUM"))

        if True:
            # ---- load constants ----
            w_ih0 = cw.tile([NI, G3], dt.float16, tag="wih0")
            nc.sync.dma_start(out=w_ih0[:], in_=d_wih0[:])
            ws = {}
            for name in ("whh0", "wih1", "whh1", "wih2", "whh2"):
                t_ = cw.tile([128, KC, G3], dt.float16, tag=name)
                nc.sync.dma_start(
                    out=t_[:], in_=d_w[name][:].rearrange("(kc p) m -> p kc m", p=128))
                ws[name] = t_
            ba = []
            bhn = []
            for l in range(3):
                tb = cw.tile([128, MT], dt.float32, tag=f"ba{l}")
                nc.sync.dma_start(out=tb[:], in_=d_ba[l][:])
                ba.append(tb)
                th = cw.tile([128, KC], dt.float32, tag=f"bhn{l}")
                nc.sync.dma_start(out=th[:], in_=d_bhn[l][:])
                bhn.append(th)
            bhnb = []
            for l in range(3):
                tr = cw.tile([128, KC, 32], dt.float32, tag=f"bhnb{l}")
                nc.sync.dma_start(out=tr[:], in_=d_bhnb[l][:])
                bhnb.append(tr)
            w_heads = cw.tile([128, KC, 4], dt.float16, tag="wheads")
            nc.sync.dma_start(
                out=w_heads[:], in_=d_heads[:].rearrange("(kc p) o -> p kc o", p=128))

            # rolling chunk handles
            h0c, h1c, h2c = {}, {}, {}

            def gi_bulk(wt, rhs_tile, gi_tile, bias_tile, kcs):
                """gi_tile[:, m, :] = bias[:, m] + sum_kc wt[:,kc,m*128..].T @ rhs"""
                for m in range(MT):
                    ps = ps_blk.tile([128, COLS], dt.float32, tag="blk")
                    for k in range(kcs):
                        if kcs == 1:
                            lhsT = wt[:, m * 128:(m + 1) * 128]
                            rhs = rhs_tile[:, :]
                        else:
                            lhsT = wt[:, k, m * 128:(m + 1) * 128]
                            rhs = rhs_tile[:, k, :]
                        nc.tensor.matmul(ps[:], lhsT, rhs,
                                         start=(k == 0), stop=(k == kcs - 1))
                    if m % 2 == 0:
                        nc.vector.tensor_scalar_add(
                            out=gi_tile[:, m, :], in0=ps[:],
                            scalar1=bias_tile[:, m:m + 1])
                    else:
                        nc.scalar.activation(out=gi_tile[:, m, :], in_=ps[:],
                                             func=AF.Identity,
                                             bias=bias_tile[:, m:m + 1])

            def gru_step(wname, bhn_tile, bhn_bc, gi_tile, lt, rhs_src, rhs_lt,
                         hout_tile):
                """One GRU step (N=32 cols). rhs_src/rhs_lt: fp16 h_prev tile+col
                (None for a fresh step, h_prev = 0). Writes fp16 h into
                hout_tile[:, :, lt*32:...]."""
                gi_rz = gi_tile[:, 0:8, lt * 32:(lt + 1) * 32]
                gi_n = gi_tile[:, 8:12, lt * 32:(lt + 1) * 32]
                fresh = rhs_src is None
                if not fresh:
                    hprev = rhs_src[:, :, rhs_lt * 32:(rhs_lt + 1) * 32]
                    prz = ps_rz.tile([128, 8, 32], dt.float32, tag="rz")
                    pn = ps_n.tile([128, 4, 32], dt.float32, tag="n")
                    # m-major: accumulation groups must stay contiguous per
                    # PSUM region (interleaved groups accumulate incorrectly)
                    for m in range(MT):
                        out = prz[:, m, :] if m < 8 else pn[:, m - 8, :]
                        for k in range(KC):
                            rhs = rhs_src[:, k, rhs_lt * 32:(rhs_lt + 1) * 32]
                            nc.tensor.matmul(out, ws[wname][:, k, m * 128:(m + 1) * 128],
                                             rhs, start=(k == 0),
                                             stop=(k == KC - 1))
                rzs = sgp.tile([128, 8, 32], dt.float32, tag="rzs")
                omz = sgp.tile([128, 4, 32], dt.float32, tag="omz")
                if fresh:
                    nc.scalar.activation(out=rzs[:], in_=gi_rz, func=AF.Sigmoid)
                    nc.scalar.activation(out=omz[:],
                                         in_=gi_tile[:, 4:8, lt * 32:(lt + 1) * 32],
                                         func=AF.Sigmoid, scale=-1.0)
                else:
                    trz = rzp.tile([128, 8, 32], dt.float32, tag="trz")
                    nc.vector.tensor_add(out=trz[:], in0=prz[:], in1=gi_rz)
                    nc.scalar.activation(out=rzs[:], in_=trz[:], func=AF.Sigmoid)
                    nc.scalar.activation(out=omz[:], in_=trz[:, 4:8, :],
                                         func=AF.Sigmoid, scale=-1.0)
                if not fresh:
                    # z * h_prev off the critical path on gpsimd
                    zh = gp.tile([128, 4, 32], dt.float32, tag="zh")
                    nc.gpsimd.tensor_mul(out=zh[:], in0=rzs[:, 4:8, :], in1=hprev)
                t3 = gp.tile([128, 4, 32], dt.float32, tag="t3")
                if fresh:
                    for j in range(KC):
                        nc.vector.scalar_tensor_tensor(
                            out=t3[:, j, :], in0=rzs[:, j, :],
                            scalar=bhn_tile[:, j:j + 1],
                            in1=gi_tile[:, 8 + j, lt * 32:(lt + 1) * 32],
                            op0=OP.mult, op1=OP.add)
                else:
                    tn = gp.tile([128, 4, 32], dt.float32, tag="tn")
                    nc.vector.tensor_add(out=tn[:], in0=pn[:], in1=bhn_bc[:])
                    tn2 = gp.tile([128, 4, 32], dt.float32, tag="tn2")
                    nc.vector.tensor_mul(out=tn2[:], in0=tn[:], in1=rzs[:, 0:4, :])
                    nc.vector.tensor_add(out=t3[:], in0=tn2[:], in1=gi_n)
                nt = gp.tile([128, 4, 32], dt.float32, tag="nt")
                nc.scalar.activation(out=nt[:], in_=t3[:], func=AF.Tanh)
                hcol = hout_tile[:, :, lt * 32:(lt + 1) * 32]
                if not fresh:
                    u = gp.tile([128, 4, 32], dt.float32, tag="u")
                    nc.vector.tensor_mul(out=u[:], in0=nt[:], in1=omz[:])
                    nc.vector.tensor_add(out=hcol, in0=u[:], in1=zh[:])
                else:
                    nc.vector.tensor_mul(out=hcol, in0=nt[:], in1=omz[:])

            def parallel_gates(gi_tile, bhn_tile, hout_tile):
                """Bulk h = (1-z)*n for all CH steps of a chunk (h_prev = 0)."""
                rzb = bigp.tile([128, 8, COLS], dt.float16, tag="rzb")
                nc.scalar.activation(out=rzb[:], in_=gi_tile[:, 0:8, :], func=AF.Sigmoid)
                t2b = bigp.tile([128, 4, COLS], dt.float16, tag="t2b")
                for j in range(KC):
                    nc.vector.scalar_tensor_tensor(
                        out=t2b[:, j, :], in0=rzb[:, j, :],
                        scalar=bhn_tile[:, j:j + 1], in1=gi_tile[:, 8 + j, :],
                        op0=OP.mult, op1=OP.add)
                nb = bigp.tile([128, 4, COLS], dt.float16, tag="nb")
                nc.scalar.activation(out=nb[:], in_=t2b[:], func=AF.Tanh)
                znb = bigp.tile([128, 4, COLS], dt.float16, tag="znb")
                nc.gpsimd.tensor_mul(out=znb[:], in0=rzb[:, 4:8, :], in1=nb[:])
                nc.vector.tensor_sub(out=hout_tile[:], in0=nb[:], in1=znb[:])

            def head_pass(col, rhs_tile, stage, row):
                """stats row = w_heads[:, col] . rhs  (M=1 matmul, 4 K-chunks)."""
                ps = ps_st.tile([1, COLS], dt.float32, tag="st")
                for k in range(KC):
                    nc.tensor.matmul(ps[:], w_heads[:, k, col:col + 1],
                                     rhs_tile[:, k, :], start=(k == 0), stop=(k == KC - 1))
                nc.scalar.activation(out=stage[:, row, :], in_=ps[:], func=AF.Copy)

            # ================= main pipeline =================
            for slot in range(NCH + 2):
                c = slot
                # ---- gi0(c) + layer-0 recurrence chunk c ----
                if c < NCH:
                    xt = xpool.tile([NI, COLS], dt.float16, tag="xt")
                    nc.sync.dma_start(out=xt[:], in_=d_xt[:, c * COLS:(c + 1) * COLS])
                    gi0 = gi0p.tile([128, MT, COLS], dt.float16, tag="gi0")
                    gi_bulk(w_ih0, xt, gi0, ba[0], 1)
                    hc = h0p.tile([128, KC, COLS], dt.float16, tag="h0c")
                    h0c[c] = hc
                    for lt in range(CH):
                        t = c * CH + lt
                        if t == 0:
                            rhs_src, rhs_lt = None, 0
                        elif lt == 0:
                            rhs_src, rhs_lt = h0c[c - 1], CH - 1
                        else:
                            rhs_src, rhs_lt = hc, lt - 1
                        gru_step("whh0", bhn[0], bhnb[0], gi0, lt,
                                 rhs_src, rhs_lt, hc)

                # ---- layer 1 on chunk c-1 ----
                c1 = slot - 1
                if 0 <= c1 < NCH:
                    gi1 = gi12p.tile([128, MT, COLS], dt.float16, tag="gi12")
                    gi_bulk(ws["wih1"], h0c[c1], gi1, ba[1], KC)
                    hc = h1p.tile([128, KC, COLS], dt.float16, tag="h1c")
                    h1c[c1] = hc
                    parallel_gates(gi1, bhn[1], hc)
                    # chain steps t % 7 == 0 (t>0) in this chunk: lt = 0, 7
                    for lt in (0, 7):
                        t = c1 * CH + lt
                        if t == 0:
                            continue
                        tp = t - DIL1
                        cp, lp = tp // CH, tp % CH
                        gru_step("whh1", bhn[1], bhnb[1], gi1, lt,
                                 h1c[cp], lp, hc)

                # ---- layer 2 + stats on chunk c-2 ----
                c2 = slot - 2
                if 0 <= c2 < NCH:
                    gi2 = gi12p.tile([128, MT, COLS], dt.float16, tag="gi12")
                    gi_bulk(ws["wih2"], h1c[c2], gi2, ba[2], KC)
                    hc = h2p.tile([128, KC, COLS], dt.float16, tag="h2c")
                    h2c[c2] = hc
                    parallel_gates(gi2, bhn[2], hc)
                    if c2 % 2 == 0 and c2 > 0:
                        t = c2 * CH  # t = 28k, lt = 0
                        tp = t - DIL2
                        cp, lp = tp // CH, tp % CH
                        gru_step("whh2", bhn[2], bhnb[2], gi2, 0,
                                 h2c[cp], lp, hc)
                    # stats / heads
                    stage = stp.tile([1, 5, COLS], dt.float32, tag="stage")
                    head_pass(0, h0c[c2], stage, 0)   # res  . h0
                    head_pass(1, h1c[c2], stage, 1)   # day  . h1
                    head_pass(2, hc, stage, 2)        # week . h2
                    head_pass(3, hc, stage, 3)        # ones . h2
                    sq = sqp.tile([128, KC, COLS], dt.float16, tag="sq")
                    nc.gpsimd.tensor_mul(out=sq[:], in0=hc[:], in1=hc[:])
                    head_pass(3, sq, stage, 4)        # ones . h2^2
                    nc.sync.dma_start(out=d_stats[:, c2 * COLS:(c2 + 1) * COLS],
                                      in_=stage[:])

    nc.compile()
    global _LAST_NC
    _LAST_NC = nc
    if globals().get("_TSIM"):
        return None

    # ---- build a cached jitted executor (mirrors bass2jax.run_bass_via_pjrt,
    # but the jit closure is constructed once so repeat calls skip retracing) ----
    import jax
    from jax.experimental.shard_map import shard_map
    from jax.sharding import Mesh, PartitionSpec, NamedSharding
    from concourse.bass2jax import (_bass_exec_p, install_neuronx_cc_hook,
                                    partition_id_tensor)

    install_neuronx_cc_hook()
    partition_name = nc.partition_id_tensor.name if nc.partition_id_tensor else None
    in_names, out_names, out_avals, zero_outs = [], [], [], []
    for alloc in nc.m.functions[0].allocations:
        if not isinstance(alloc, mybir.MemoryLocationSet):
            continue
        name = alloc.memorylocations[0].name
        if alloc.kind == "ExternalInput":
            if name != partition_name:
                in_names.append(name)
        elif alloc.kind == "ExternalOutput":
            out_names.append(name)
            shape = tuple(alloc.tensor_shape)
            npdt = mybir.dt.np(alloc.dtype)
            out_avals.append(jax.core.ShapedArray(shape, npdt))
            zero_outs.append(np.zeros(shape, npdt))
    n_params, n_outs = len(in_names), len(out_names)
    all_names = in_names + out_names + ([partition_name] if partition_name else [])

    def _body(*args):
        operands = list(args)
        if partition_name is not None:
            operands.append(partition_id_tensor())
        outs = _bass_exec_p.bind(
            *operands, out_avals=tuple(out_avals), in_names=tuple(all_names),
            out_names=tuple(out_names), lowering_input_output_aliases=(),
            sim_require_finite=True, sim_require_nnan=True, nc=nc)
        return tuple(outs)

    try:
        devices = jax.devices("axon")[:NCORES]
    except Exception:
        devices = jax.devices()[:NCORES]
    assert len(devices) == NCORES, f"need {NCORES} neuron cores, got {devices}"
    mesh = Mesh(np.asarray(devices), ("core",))
    sharding = NamedSharding(mesh, PartitionSpec("core"))
    donate = tuple(range(n_params, n_params + n_outs))
    sharded = jax.jit(
        shard_map(_body, mesh=mesh,
                  in_specs=(PartitionSpec("core"),) * (n_params + n_outs),
                  out_specs=(PartitionSpec("core"),) * n_outs,
                  check_rep=False),
        donate_argnums=donate, keep_unused=True)

    def _concat_inputs(in_maps):
        return [np.concatenate([np.asarray(m[name]) for m in in_maps], axis=0)
                for name in in_names]

    def _zeros():
        return [np.zeros((NCORES * z.shape[0], *z.shape[1:]), z.dtype)
                for z in zero_outs]

    def run(in_maps):
        out_arrs = sharded(*_concat_inputs(in_maps), *_zeros())
        stats = np.asarray(out_arrs[out_names.index("stats")])
        return list(stats.reshape(NCORES, 5, COLS_T))

    def time_it(in_maps, iters=10):
        import time
        dev_in = [jax.device_put(a, sharding) for a in _concat_inputs(in_maps)]
        jax.block_until_ready(sharded(*dev_in, *_zeros()))  # warm
        times = []
        for _ in range(iters):
            zs = _zeros()
            t0 = time.perf_counter()
            jax.block_until_ready(sharded(*dev_in, *zs))
            times.append(time.perf_counter() - t0)
        return times

    run.time_it = time_it
    return run


def _prep_inputs(inputs):
    f16 = np.float16
    x = np.asarray(inputs["x"], np.float32)          # [256, 168, 64]
    maps = []
    shared = {}
    for l in range(3):
        wih = np.asarray(inputs[f"W_ih{l}"], np.float32)   # [1536, in]
        whh = np.asarray(inputs[f"W_hh{l}"], np.float32)   # [1536, 512]
        bih = np.asarray(inputs[f"b_ih{l}"], np.float32)
        bhh = np.asarray(inputs[f"b_hh{l}"], np.float32)
        shared[f"wih{l}" if l else "wih0"] = np.ascontiguousarray(wih.T).astype(f16)
        shared[f"whh{l}"] = np.ascontiguousarray(whh.T).astype(f16)      # [512,1536]
        bias_a = bih.copy()
        bias_a[:2 * H] += bhh[:2 * H]          # rz tiles: b_ih + b_hh ; n tiles: b_ih
        shared[f"ba{l}"] = np.ascontiguousarray(bias_a.reshape(MT, 128).T)  # [128,12]
        shared[f"bhn{l}"] = np.ascontiguousarray(bhh[2 * H:].reshape(KC, 128).T)
        shared[f"bhnb{l}"] = np.ascontiguousarray(
            np.broadcast_to(bhh[2 * H:].reshape(KC, 128).T[:, :, None],
                            (128, KC, 32)).astype(np.float32))
    heads = np.zeros((H, 4), np.float32)
    heads[:, 0] = np.asarray(inputs["w_res"], np.float32)[0]
    heads[:, 1] = np.asarray(inputs["w_day"], np.float32)[0]
    heads[:, 2] = np.asarray(inputs["w_week"], np.float32)[0]
    heads[:, 3] = 1.0
    shared["heads"] = heads.astype(f16)

    for k in range(NCORES):
        xc = x[k * BC:(k + 1) * BC]                        # [32, 168, 64]
        xt = np.ascontiguousarray(xc.transpose(2, 1, 0).reshape(NI, COLS_T))
        m = dict(shared)
        m["xt"] = xt.astype(f16)
        maps.append(m)
    return maps


def kernel(**inputs):
    global _RUNNER
    if _RUNNER is None:
        _RUNNER = _build_runner()
    maps = _prep_inputs(inputs)
    stats = _RUNNER(maps)          # list of [5, 5376] per core

    b_res = float(np.asarray(inputs["b_res"], np.float32)[0])
    b_day = float(np.asarray(inputs["b_day"], np.float32)[0])
    b_week = float(np.asarray(inputs["b_week"], np.float32)[0])

    outputs = np.zeros((B_FULL, T), np.float32)
    aux = 0.0
    for k in range(NCORES):
        s = np.asarray(stats[k], np.float64)               # [5, 5376]
        res, day, week, sm, sq = [s[i].reshape(T, BC) for i in range(5)]
        out = np.tanh(res + b_res) + np.tanh(day + b_day) + np.tanh(week + b_week)
        outputs[k * BC:(k + 1) * BC, :] = out.T.astype(np.float32)
        var = (sq - sm * sm / H) / (H - 1)
        aux += var.sum()
    aux = np.float32(aux / B_FULL)
    return outputs, aux
